# revision 16
# baseline (speedup 1.0000x reference)
"""Multi-head attention TRN2 kernel (B=4, S=2048, E=128, H=8) on 8 NeuronCores.

Sharding: core c handles batch b = c // 2 and head group g = c % 2
(heads 4g .. 4g+3).  Each core computes the partial output
outT_partial[g_out, s] = sum_{h in group} (softmax(QK^T/sqrt(E)) V)_h @ Wo_h
for its batch, transposed.  Host sums the two head-group partials per batch,
transposes, and adds bo (+ bv @ Wo, folded out of the device kernel).

Device algorithm (fused projections):
  A_h   = Wq_h Wk_h^T               [e, e']   (scores bilinear form)
  u_h   = Wk_h bq_h                 [e']      (the only softmax-variant bias
                                               term; q.bk / bq.bk terms are
                                               constant over keys t, hence
                                               softmax-invariant and dropped)
  P_h   = A_h^T-contracted qT + u   [e', s]   (one proj replaces Q and K)
  scoresT[t, s] = qT_blk^T @ P_h    (t on partitions)
  attnT = exp(scale*scoresT)        (13/16 blocks on ACT engine, 3/16 via a
                                     Schraudolph int16-bitcast approx on DVE)
  Wvo_h = Wv_h Wo_h                 [e, g]    (output proj folded into V)
  VW    = qT_blk^T @ Wvo            [t, (h g)]
  YT_h  = sum_t VW_blk^T @ attnT    [g, s]    (PSUM accum over 16 t-blocks)
  denom = ones^T @ (folded attnT)   (column sums; unbalanced fold tree:
                                     Pool adds f1, DVE f2..f4, then a 2-block
                                     accumulating ones-matmul)
  outT += YT_h * (1/denom)
"""

import sys

for _p in ("/opt/trn_rl_repo",):
    if _p not in sys.path:
        sys.path.insert(0, _p)

import numpy as np

import concourse.bass as bass
import concourse.mybir as mybir
import concourse.tile as tile
from concourse.bass_utils import run_bass_kernel_spmd
from concourse.masks import make_identity

F32 = mybir.dt.float32
F16 = mybir.dt.float16
I16 = mybir.dt.int16

B, S, E, H = 4, 2048, 128, 8
NH = 4          # heads per core
TB = S // 128   # 16 t blocks
S_SPLIT = 2     # s-direction split per head (pipelining unit)
SW = S // S_SPLIT        # 1024
NC = 512                 # psum-bank chunk (fp32)
SCALE = 1.0 / np.sqrt(E)

# Schraudolph fp16 exp on DVE: i16 = round(score*SCH_A + SCH_B) bitcast fp16.
# SCH_A = SCALE * 2^10/ln2; SCH_B = 15*1024 - 59 (C=59 minimizes rel RMS).
SCH_A = float(SCALE * (2.0**10) / np.log(2.0))
SCH_B = 15360.0 - 59.0
SCHRAUD = (3, 6, 11)  # t-blocks approximated on DVE (off exp critical paths)

AV_SKEW = 4     # AV matmul for t-block tb emitted at loop step tb+AV_SKEW

_prog_cache = {}


def build_program():
    if "nc" in _prog_cache:
        return _prog_cache["nc"]

    import concourse.bacc as bacc

    nc = bacc.Bacc("TRN2", target_bir_lowering=False, debug=False)

    q_d = nc.dram_tensor("q", [S, E], F32, kind="ExternalInput").ap()
    wq_d = nc.dram_tensor("Wq", [NH, E, E], F32, kind="ExternalInput").ap()
    wk_d = nc.dram_tensor("Wk", [NH, E, E], F32, kind="ExternalInput").ap()
    wv_d = nc.dram_tensor("Wv", [NH, E, E], F32, kind="ExternalInput").ap()
    wo_d = nc.dram_tensor("Wo", [NH, E, E], F32, kind="ExternalInput").ap()
    bq_d = nc.dram_tensor("bq", [NH, E], F32, kind="ExternalInput").ap()
    out_d = nc.dram_tensor("out", [E, S], F32, kind="ExternalOutput").ap()

    with tile.TileContext(nc) as tc:
        _emit(nc, tc, q_d, wq_d, wk_d, wv_d, wo_d, bq_d, out_d)

    nc.compile()
    _prog_cache["nc"] = nc
    return nc


def _emit(nc, tc, q_d, wq_d, wk_d, wv_d, wo_d, bq_d, out_d):
    from contextlib import ExitStack

    MULT = mybir.AluOpType.mult
    ADD = mybir.AluOpType.add

    ctx = ExitStack()
    consts = ctx.enter_context(tc.tile_pool(name="consts", bufs=1))
    attns = ctx.enter_context(tc.tile_pool(name="attns", bufs=2))
    folds = ctx.enter_context(tc.tile_pool(name="folds", bufs=1))
    works = ctx.enter_context(tc.tile_pool(name="works", bufs=2))
    # PSUM budget (16KB/partition): sc 2x4KB + y 4x2KB (y also serves the
    # startup/projection transients, so tails decouple from the next loop)
    psc = ctx.enter_context(tc.tile_pool(name="psc", bufs=2, space="PSUM"))
    psy = ctx.enter_context(tc.tile_pool(name="psy", bufs=4, space="PSUM"))

    # ---- constants ----
    ident = consts.tile([128, 128], F32, tag="ident")
    make_identity(nc, ident)
    ones16 = consts.tile([128, 128], F16, tag="ones")
    nc.vector.memset(ones16, 1.0)

    # ---- loads ----
    q_sb = consts.tile([128, TB, 128], F32, tag="qsb")  # [s_sub, s_blk, e]
    q_r = q_d.rearrange("(sb p) e -> p sb e", p=128)
    for qc in range(4):
        nc.sync.dma_start(
            out=q_sb[:, qc * (TB // 4) : (qc + 1) * (TB // 4), :],
            in_=q_r[:, qc * (TB // 4) : (qc + 1) * (TB // 4), :],
        )
    w_stage = consts.tile([128, 4, NH, 128], F32, tag="wstage")  # [i, w, h, j]
    for i, wd in enumerate((wq_d, wk_d, wv_d, wo_d)):
        nc.sync.dma_start(out=w_stage[:, i], in_=wd.rearrange("h i j -> i h j"))
    bq_sb = consts.tile([128, NH], F32, tag="bq")  # [f, h]
    nc.sync.dma_start(out=bq_sb, in_=bq_d.rearrange("h f -> f h"))
    bq16 = consts.tile([128, NH], F16, tag="bq16")
    nc.vector.tensor_copy(bq16, bq_sb)

    # ---- qT via PE transposes ----
    qT = consts.tile([128, TB, 128], F16, tag="qT")  # [e, s_blk, s_sub]
    for sb in range(TB):
        pt = psy.tile([128, 128], F32, tag="y", name=f"pt_{sb}")
        nc.tensor.transpose(pt, q_sb[:, sb, :], ident)
        nc.scalar.copy(qT[:, sb, :], pt)
    qT_flat = qT.rearrange("e sb p -> e (sb p)")  # [e, s]

    # ---- weight transposes (WqT, WkT, WvT as [f, h, e] fp16) ----
    wqT = consts.tile([128, NH, 128], F16, tag="wqT")
    wkT = consts.tile([128, NH, 128], F16, tag="wkT")
    wvT = consts.tile([128, NH, 128], F16, tag="wvT")
    for wi, wt in ((0, wqT), (1, wkT), (2, wvT)):
        for h in range(NH):
            pw = psy.tile([128, 128], F32, tag="y", name=f"pw_{wi}_{h}")
            nc.tensor.transpose(pw, w_stage[:, wi, h, :], ident)
            nc.scalar.copy(wt[:, h, :], pw)
    wo16 = consts.tile([128, NH, 128], F16, tag="wo16")  # [f, h, g]
    nc.scalar.copy(wo16, w_stage[:, 3])

    # ---- fused weights: A = Wq Wk^T, u = Wk bq, Wvo = Wv Wo ----
    A_sb = consts.tile([128, NH, 128], F16, tag="A")    # [i, h, j]
    u_sb = consts.tile([128, NH], F32, tag="u")         # [j, h]
    wvo = consts.tile([128, NH, 128], F16, tag="wvo")   # [e, h, g]
    for h in range(NH):
        pa = psy.tile([128, 128], F32, tag="y", name=f"pa_{h}")
        nc.tensor.matmul(pa, lhsT=wqT[:, h, :], rhs=wkT[:, h, :], start=True, stop=True)
        nc.scalar.copy(A_sb[:, h, :], pa)
        pv = psy.tile([128, 128], F32, tag="y", name=f"pv_{h}")
        nc.tensor.matmul(pv, lhsT=wvT[:, h, :], rhs=wo16[:, h, :], start=True, stop=True)
        nc.scalar.copy(wvo[:, h, :], pv)
    for h in range(NH):
        pu = psy.tile([128, 1], F32, tag="y", name=f"pu_{h}")
        nc.tensor.matmul(pu, lhsT=wkT[:, h, :], rhs=bq16[:, h : h + 1], start=True, stop=True)
        nc.scalar.copy(u_sb[:, h : h + 1], pu)

    # ---- VW = qT_blk^T @ Wvo (all heads at once) ----
    vw = consts.tile([128, TB, NH, 128], F16, tag="vw")  # [t_sub, tb, h, g]
    vw_flat = vw.rearrange("p t h g -> p (t h g)")
    wvo_flat = wvo.rearrange("e h g -> e (h g)")  # [e, 512]
    for tb in range(TB):
        pvw = psy.tile([128, NC], F32, tag="y", name=f"pvw_{tb}")
        nc.tensor.matmul(pvw, lhsT=qT[:, tb, :], rhs=wvo_flat, start=True, stop=True)
        nc.vector.tensor_copy(vw_flat[:, tb * NC : (tb + 1) * NC], pvw)

    # ---- P projections (all heads upfront; h>0 deferred as PE chores) ----
    P_all = consts.tile([128, NH, S], F16, tag="P")  # [j, h, s]

    def p_chunk(h, j):
        def emit():
            pp = psy.tile([128, NC], F32, tag="y", name=f"pp_{h}_{j}")
            nc.tensor.matmul(
                pp, lhsT=A_sb[:, h, :], rhs=qT_flat[:, j * NC : (j + 1) * NC],
                start=True, stop=True,
            )
            nc.vector.tensor_scalar_add(
                P_all[:, h, j * NC : (j + 1) * NC], pp, u_sb[:, h : h + 1]
            )
        return emit

    for j in range(4):
        p_chunk(0, j)()
    chores = [p_chunk(h, j) for h in range(1, NH) for j in range(4)]

    # ---- output accumulators (ping-pong per s-half across heads) ----
    acc_a = [
        consts.tile([128, SW], F32, tag=f"acc_a{sh}", name=f"acc_a{sh}")
        for sh in range(S_SPLIT)
    ]
    acc_b = [
        consts.tile([128, SW], F32, tag=f"acc_b{sh}", name=f"acc_b{sh}")
        for sh in range(S_SPLIT)
    ]

    pending = []  # deferred per-head-half tails

    for h in range(NH):
        for sh in range(S_SPLIT):
            s0 = sh * SW
            attnT = attns.tile(
                [128, TB, SW], F16, tag="attnT", name=f"attnT_{h}_{sh}"
            )  # [t_sub, t_blk, s]
            f1 = folds.tile([128, 7, SW], F16, tag="f1", name=f"f1_{h}_{sh}")
            f2 = folds.tile([128, 3, SW], F16, tag="f2", name=f"f2_{h}_{sh}")
            f3 = folds.tile([128, 2, SW], F16, tag="f3", name=f"f3_{h}_{sh}")
            f4 = folds.tile([128, 1, SW], F16, tag="f4", name=f"f4_{h}_{sh}")
            fl = folds.tile([128, 2, SW], F16, tag="fl", name=f"fl_{h}_{sh}")
            ys = [
                psy.tile([128, NC], F32, tag="y", name=f"y_{h}_{sh}_{c}")
                for c in range(2)
            ]

            def emit_av(tb, h=h, ys=ys, attnT=attnT):
                for c in range(2):
                    nc.tensor.matmul(
                        ys[c],
                        lhsT=vw[:, tb, h, :],
                        rhs=attnT[:, tb, c * NC : (c + 1) * NC],
                        start=(tb == 0),
                        stop=(tb == TB - 1),
                    )

            for tb in range(TB):
                # ---- AV first (deps long satisfied): absorbs the wait for
                #      exp(tb-2) to free the sc slot ----
                if tb >= AV_SKEW:
                    emit_av(tb - AV_SKEW)
                # ---- scores ----
                sc = psc.tile([128, SW], F32, tag="sc", name=f"sc_{h}_{sh}_{tb}")
                for j in range(2):
                    nc.tensor.matmul(
                        sc[:, j * NC : (j + 1) * NC],
                        lhsT=qT[:, tb, :],
                        rhs=P_all[:, h, s0 + j * NC : s0 + (j + 1) * NC],
                        start=True,
                        stop=True,
                    )
                # ---- exp (ACT or DVE-Schraudolph) ----
                if tb in SCHRAUD:
                    nc.vector.tensor_scalar(
                        attnT[:, tb, :].bitcast(I16), sc, SCH_A, SCH_B, MULT, ADD
                    )
                else:
                    nc.scalar.activation(
                        attnT[:, tb, :], sc, mybir.ActivationFunctionType.Exp,
                        scale=SCALE,
                    )
                # ---- deferred tail of previous head-half / startup chores ----
                if tb == 5 and pending:
                    pending.pop(0)()
                if tb in (4, 8) and chores:
                    chores.pop(0)()
                    chores.pop(0)()
                # ---- fold tree over blocks 0..13 (14/15 deliberately left
                #      out: they join at the dns accumulation, keeping the
                #      end-of-loop critical chain short) ----
                if 7 <= tb <= 11:
                    i = tb - 7
                    nc.gpsimd.tensor_add(
                        f1[:, i, :], attnT[:, i, :], attnT[:, tb, :]
                    )
                if tb == 11:
                    nc.gpsimd.tensor_add(f2[:, 0, :], f1[:, 0, :], f1[:, 4, :])
                if tb == 12:
                    nc.vector.tensor_add(f1[:, 5, :], attnT[:, 5, :], attnT[:, 12, :])
                    nc.vector.tensor_add(f2[:, 1, :], f1[:, 1, :], f1[:, 5, :])
                    nc.vector.tensor_add(f3[:, 1, :], f2[:, 1, :], f1[:, 3, :])
                if tb == 13:
                    nc.vector.tensor_add(f1[:, 6, :], attnT[:, 6, :], attnT[:, 13, :])
                    nc.vector.tensor_add(f2[:, 2, :], f1[:, 2, :], f1[:, 6, :])
                    nc.vector.tensor_add(f3[:, 0, :], f2[:, 0, :], f2[:, 2, :])
                if tb == 14:
                    nc.vector.tensor_add(f4[:, 0, :], f3[:, 0, :], f3[:, 1, :])
            for tb in range(TB - AV_SKEW, TB):
                emit_av(tb)

            def make_tail(h=h, sh=sh, s0=s0, ys=ys, f4=f4, fl=fl):
                def tail():
                    # denominator: ones-matmul over f4 (blocks 0..14) + block 15
                    dns = psc.tile([128, SW], F32, tag="sc", name=f"dns_{h}_{sh}")
                    for c in range(2):
                        csl = slice(c * NC, (c + 1) * NC)
                        nc.tensor.matmul(
                            dns[:, csl], lhsT=ones16, rhs=f4[:, 0, csl],
                            start=True, stop=False,
                        )
                        nc.tensor.matmul(
                            dns[:, csl], lhsT=ones16, rhs=attnT[:, TB - 2, csl],
                            start=False, stop=False,
                        )
                        nc.tensor.matmul(
                            dns[:, csl], lhsT=ones16, rhs=attnT[:, TB - 1, csl],
                            start=False, stop=True,
                        )
                    for c in range(2):
                        csl = slice(c * NC, (c + 1) * NC)
                        osl = slice(s0 + c * NC, s0 + (c + 1) * NC)
                        recip = works.tile(
                            [128, NC], F32, tag="recip", name=f"recip_{h}_{sh}_{c}"
                        )
                        nc.vector.reciprocal_approx_fast(recip, dns[:, csl])
                        if h == 0:
                            nc.vector.tensor_mul(acc_a[sh][:, csl], ys[c], recip)
                            continue
                        ynorm = works.tile(
                            [128, NC], F32, tag="ynorm", name=f"yn_{h}_{sh}_{c}"
                        )
                        nc.vector.tensor_mul(ynorm, ys[c], recip)
                        if h == 1:
                            nc.gpsimd.tensor_add(
                                acc_b[sh][:, csl], acc_a[sh][:, csl], ynorm
                            )
                        elif h == 2:
                            nc.gpsimd.tensor_add(
                                acc_a[sh][:, csl], acc_b[sh][:, csl], ynorm
                            )
                        else:
                            osb = works.tile(
                                [128, NC], F32, tag="osb", name=f"osb_{h}_{sh}_{c}"
                            )
                            nc.gpsimd.tensor_add(osb, acc_a[sh][:, csl], ynorm)
                            nc.sync.dma_start(out=out_d[:, osl], in_=osb)
                return tail

            pending.append(make_tail())
    while pending:
        pending.pop(0)()

    ctx.close()


def _in_maps(inputs):
    q = np.asarray(inputs["q"], dtype=np.float32)
    Wq = np.asarray(inputs["Wq"], dtype=np.float32)
    bq = np.asarray(inputs["bq"], dtype=np.float32)
    Wk = np.asarray(inputs["Wk"], dtype=np.float32)
    Wv = np.asarray(inputs["Wv"], dtype=np.float32)
    Wo = np.asarray(inputs["Wo"], dtype=np.float32).reshape(H, E, E)
    maps = []
    for c in range(8):
        b = c // 2
        hs = slice(4 * (c % 2), 4 * (c % 2) + 4)
        maps.append(
            {
                "q": np.ascontiguousarray(q[b]),
                "Wq": np.ascontiguousarray(Wq[hs]),
                "Wk": np.ascontiguousarray(Wk[hs]),
                "Wv": np.ascontiguousarray(Wv[hs]),
                "Wo": np.ascontiguousarray(Wo[hs]),
                "bq": np.ascontiguousarray(bq[hs]),
            }
        )
    return maps


def kernel(**inputs):
    nc = build_program()
    maps = _in_maps(inputs)
    res = run_bass_kernel_spmd(nc, maps, core_ids=list(range(8)))
    bo = np.asarray(inputs["bo"], dtype=np.float32)
    bv = np.asarray(inputs["bv"], dtype=np.float32)
    Wo = np.asarray(inputs["Wo"], dtype=np.float32).reshape(H, E, E)
    # V-bias contribution folded out of the device kernel:
    # sum_h softmax(..)@ (qWv + bv) @ Wo_h = device_partials + sum_h bv_h @ Wo_h
    bo_eff = bo + np.einsum("he,hef->f", bv, Wo).astype(np.float32)
    out = np.empty((B, S, E), dtype=np.float32)
    for b in range(B):
        part = res.results[2 * b]["out"] + res.results[2 * b + 1]["out"]
        out[b] = part.T + bo_eff
    return out


# revision 17
# speedup vs baseline: 1.0272x; 1.0272x over previous
"""Multi-head attention TRN2 kernel (B=4, S=2048, E=128, H=8) on 8 NeuronCores.

Sharding: core c handles batch b = c // 2 and head group g = c % 2
(heads 4g .. 4g+3).  Each core computes the partial output
outT_partial[g_out, s] = sum_{h in group} (softmax(QK^T/sqrt(E)) V)_h @ Wo_h
for its batch, transposed.  Host sums the two head-group partials per batch,
transposes, and adds bo (+ bv @ Wo, folded out of the device kernel).

Device algorithm (fused projections):
  A_h   = Wq_h Wk_h^T               [e, e']   (scores bilinear form)
  u_h   = Wk_h bq_h                 [e']      (the only softmax-variant bias
                                               term; q.bk / bq.bk terms are
                                               constant over keys t, hence
                                               softmax-invariant and dropped)
  P_h   = A_h^T-contracted qT + u   [e', s]   (one proj replaces Q and K)
  scoresT[t, s] = qT_blk^T @ P_h    (t on partitions)
  attnT = exp(scale*scoresT)        (13/16 blocks on ACT engine, 3/16 via a
                                     Schraudolph int16-bitcast approx on DVE)
  Wvo_h = Wv_h Wo_h                 [e, g]    (output proj folded into V)
  VW    = qT_blk^T @ Wvo            [t, (h g)]
  YT_h  = sum_t VW_blk^T @ attnT    [g, s]    (PSUM accum over 16 t-blocks)
  denom = ones^T @ (folded attnT)   (column sums; unbalanced fold tree:
                                     Pool adds f1, DVE f2..f4, then a 2-block
                                     accumulating ones-matmul)
  outT += YT_h * (1/denom)
"""

import sys

for _p in ("/opt/trn_rl_repo",):
    if _p not in sys.path:
        sys.path.insert(0, _p)

import numpy as np

import concourse.bass as bass
import concourse.mybir as mybir
import concourse.tile as tile
from concourse.bass_utils import run_bass_kernel_spmd
from concourse.masks import make_identity

F32 = mybir.dt.float32
F16 = mybir.dt.float16
I16 = mybir.dt.int16

B, S, E, H = 4, 2048, 128, 8
NH = 4          # heads per core
TB = S // 128   # 16 t blocks
S_SPLIT = 2     # s-direction split per head (pipelining unit)
SW = S // S_SPLIT        # 1024
NC = 512                 # psum-bank chunk (fp32)
SCALE = 1.0 / np.sqrt(E)

# Schraudolph fp16 exp on DVE: i16 = round(score*SCH_A + SCH_B) bitcast fp16.
# SCH_A = SCALE * 2^10/ln2; SCH_B = 15*1024 - 59 (C=59 minimizes rel RMS).
SCH_A = float(SCALE * (2.0**10) / np.log(2.0))
SCH_B = 15360.0 - 59.0
SCHRAUD = (3, 6, 11)  # t-blocks approximated on DVE (off exp critical paths)

AV_SKEW = 4     # AV matmul for t-block tb emitted at loop step tb+AV_SKEW

_prog_cache = {}


def build_program():
    if "nc" in _prog_cache:
        return _prog_cache["nc"]

    import concourse.bacc as bacc

    nc = bacc.Bacc("TRN2", target_bir_lowering=False, debug=False)

    q_d = nc.dram_tensor("q", [S, E], F32, kind="ExternalInput").ap()
    wq_d = nc.dram_tensor("Wq", [NH, E, E], F32, kind="ExternalInput").ap()
    wk_d = nc.dram_tensor("Wk", [NH, E, E], F32, kind="ExternalInput").ap()
    wv_d = nc.dram_tensor("Wv", [NH, E, E], F32, kind="ExternalInput").ap()
    wo_d = nc.dram_tensor("Wo", [NH, E, E], F32, kind="ExternalInput").ap()
    bq_d = nc.dram_tensor("bq", [NH, E], F32, kind="ExternalInput").ap()
    out_d = nc.dram_tensor("out", [E, S], F32, kind="ExternalOutput").ap()

    with tile.TileContext(nc) as tc:
        _emit(nc, tc, q_d, wq_d, wk_d, wv_d, wo_d, bq_d, out_d)

    nc.compile()
    _prog_cache["nc"] = nc
    return nc


def _emit(nc, tc, q_d, wq_d, wk_d, wv_d, wo_d, bq_d, out_d):
    from contextlib import ExitStack

    MULT = mybir.AluOpType.mult
    ADD = mybir.AluOpType.add

    ctx = ExitStack()
    consts = ctx.enter_context(tc.tile_pool(name="consts", bufs=1))
    attns = ctx.enter_context(tc.tile_pool(name="attns", bufs=2))
    folds = ctx.enter_context(tc.tile_pool(name="folds", bufs=1))
    works = ctx.enter_context(tc.tile_pool(name="works", bufs=2))
    # PSUM budget (16KB/partition): sc 2x4KB + y 4x2KB (y also serves the
    # startup/projection transients, so tails decouple from the next loop)
    psc = ctx.enter_context(tc.tile_pool(name="psc", bufs=2, space="PSUM"))
    psy = ctx.enter_context(tc.tile_pool(name="psy", bufs=4, space="PSUM"))

    # ---- constants ----
    ident = consts.tile([128, 128], F32, tag="ident")
    make_identity(nc, ident)
    ones16 = consts.tile([128, 128], F16, tag="ones")
    nc.vector.memset(ones16, 1.0)

    # ---- loads ----
    q_sb = consts.tile([128, TB, 128], F32, tag="qsb")  # [s_sub, s_blk, e]
    q_r = q_d.rearrange("(sb p) e -> p sb e", p=128)
    for qc in range(4):
        nc.sync.dma_start(
            out=q_sb[:, qc * (TB // 4) : (qc + 1) * (TB // 4), :],
            in_=q_r[:, qc * (TB // 4) : (qc + 1) * (TB // 4), :],
        )
    w_stage = consts.tile([128, 4, NH, 128], F32, tag="wstage")  # [i, w, h, j]
    for i, wd in enumerate((wq_d, wk_d, wv_d, wo_d)):
        nc.sync.dma_start(out=w_stage[:, i], in_=wd.rearrange("h i j -> i h j"))
    bq_sb = consts.tile([128, NH], F32, tag="bq")  # [f, h]
    nc.sync.dma_start(out=bq_sb, in_=bq_d.rearrange("h f -> f h"))
    bq16 = consts.tile([128, NH], F16, tag="bq16")
    nc.vector.tensor_copy(bq16, bq_sb)

    # ---- qT via PE transposes ----
    qT = consts.tile([128, TB, 128], F16, tag="qT")  # [e, s_blk, s_sub]
    for sb in range(TB):
        pt = psy.tile([128, 128], F32, tag="y", name=f"pt_{sb}")
        nc.tensor.transpose(pt, q_sb[:, sb, :], ident)
        nc.scalar.copy(qT[:, sb, :], pt)
    qT_flat = qT.rearrange("e sb p -> e (sb p)")  # [e, s]

    # ---- weight transposes (WqT, WkT, WvT as [f, h, e] fp16) ----
    wqT = consts.tile([128, NH, 128], F16, tag="wqT")
    wkT = consts.tile([128, NH, 128], F16, tag="wkT")
    wvT = consts.tile([128, NH, 128], F16, tag="wvT")
    for wi, wt in ((0, wqT), (1, wkT), (2, wvT)):
        for h in range(NH):
            pw = psy.tile([128, 128], F32, tag="y", name=f"pw_{wi}_{h}")
            nc.tensor.transpose(pw, w_stage[:, wi, h, :], ident)
            nc.scalar.copy(wt[:, h, :], pw)
    wo16 = consts.tile([128, NH, 128], F16, tag="wo16")  # [f, h, g]
    nc.scalar.copy(wo16, w_stage[:, 3])

    # ---- fused weights: A = Wq Wk^T, u = Wk bq, Wvo = Wv Wo ----
    A_sb = consts.tile([128, NH, 128], F16, tag="A")    # [i, h, j]
    u_sb = consts.tile([128, NH], F32, tag="u")         # [j, h]
    wvo = consts.tile([128, NH, 128], F16, tag="wvo")   # [e, h, g]
    for h in range(NH):
        pa = psy.tile([128, 128], F32, tag="y", name=f"pa_{h}")
        nc.tensor.matmul(pa, lhsT=wqT[:, h, :], rhs=wkT[:, h, :], start=True, stop=True)
        nc.scalar.copy(A_sb[:, h, :], pa)
        pv = psy.tile([128, 128], F32, tag="y", name=f"pv_{h}")
        nc.tensor.matmul(pv, lhsT=wvT[:, h, :], rhs=wo16[:, h, :], start=True, stop=True)
        nc.scalar.copy(wvo[:, h, :], pv)
    for h in range(NH):
        pu = psy.tile([128, 1], F32, tag="y", name=f"pu_{h}")
        nc.tensor.matmul(pu, lhsT=wkT[:, h, :], rhs=bq16[:, h : h + 1], start=True, stop=True)
        nc.scalar.copy(u_sb[:, h : h + 1], pu)

    # ---- VW = qT_blk^T @ Wvo (all heads at once) ----
    vw = consts.tile([128, TB, NH, 128], F16, tag="vw")  # [t_sub, tb, h, g]
    vw_flat = vw.rearrange("p t h g -> p (t h g)")
    wvo_flat = wvo.rearrange("e h g -> e (h g)")  # [e, 512]
    for tb in range(TB):
        pvw = psy.tile([128, NC], F32, tag="y", name=f"pvw_{tb}")
        nc.tensor.matmul(pvw, lhsT=qT[:, tb, :], rhs=wvo_flat, start=True, stop=True)
        nc.vector.tensor_copy(vw_flat[:, tb * NC : (tb + 1) * NC], pvw)

    # ---- P projections (all heads upfront; h>0 deferred as PE chores) ----
    P_all = consts.tile([128, NH, S], F16, tag="P")  # [j, h, s]

    def p_chunk(h, j):
        def emit():
            pp = psy.tile([128, NC], F32, tag="y", name=f"pp_{h}_{j}")
            nc.tensor.matmul(
                pp, lhsT=A_sb[:, h, :], rhs=qT_flat[:, j * NC : (j + 1) * NC],
                start=True, stop=True,
            )
            nc.vector.tensor_scalar_add(
                P_all[:, h, j * NC : (j + 1) * NC], pp, u_sb[:, h : h + 1]
            )
        return emit

    for j in range(4):
        p_chunk(0, j)()
    chores = [p_chunk(h, j) for h in range(1, NH) for j in range(4)]

    # ---- output accumulators (ping-pong per s-half across heads) ----
    acc_a = [
        consts.tile([128, SW], F32, tag=f"acc_a{sh}", name=f"acc_a{sh}")
        for sh in range(S_SPLIT)
    ]
    acc_b = [
        consts.tile([128, SW], F32, tag=f"acc_b{sh}", name=f"acc_b{sh}")
        for sh in range(S_SPLIT)
    ]

    pending = []  # deferred per-head-half tails

    for h in range(NH):
        for sh in range(S_SPLIT):
            s0 = sh * SW
            attnT = attns.tile(
                [128, TB, SW], F16, tag="attnT", name=f"attnT_{h}_{sh}"
            )  # [t_sub, t_blk, s]
            f1 = folds.tile([128, 7, SW], F16, tag="f1", name=f"f1_{h}_{sh}")
            f2 = folds.tile([128, 3, SW], F16, tag="f2", name=f"f2_{h}_{sh}")
            f3 = folds.tile([128, 2, SW], F16, tag="f3", name=f"f3_{h}_{sh}")
            f4 = folds.tile([128, 1, SW], F16, tag="f4", name=f"f4_{h}_{sh}")
            fl = folds.tile([128, 2, SW], F16, tag="fl", name=f"fl_{h}_{sh}")
            ys = [
                psy.tile([128, NC], F32, tag="y", name=f"y_{h}_{sh}_{c}")
                for c in range(2)
            ]

            def emit_av(tb, h=h, ys=ys, attnT=attnT):
                for c in range(2):
                    nc.tensor.matmul(
                        ys[c],
                        lhsT=vw[:, tb, h, :],
                        rhs=attnT[:, tb, c * NC : (c + 1) * NC],
                        start=(tb == 0),
                        stop=(tb == TB - 1),
                    )

            for tb in range(TB):
                # ---- AV first (deps long satisfied): absorbs the wait for
                #      exp(tb-2) to free the sc slot ----
                if tb >= AV_SKEW:
                    emit_av(tb - AV_SKEW)
                # ---- scores ----
                sc = psc.tile([128, SW], F32, tag="sc", name=f"sc_{h}_{sh}_{tb}")
                for j in range(2):
                    nc.tensor.matmul(
                        sc[:, j * NC : (j + 1) * NC],
                        lhsT=qT[:, tb, :],
                        rhs=P_all[:, h, s0 + j * NC : s0 + (j + 1) * NC],
                        start=True,
                        stop=True,
                    )
                # ---- exp (ACT or DVE-Schraudolph) ----
                if tb in SCHRAUD:
                    nc.vector.tensor_scalar(
                        attnT[:, tb, :].bitcast(I16), sc, SCH_A, SCH_B, MULT, ADD
                    )
                else:
                    nc.scalar.activation(
                        attnT[:, tb, :], sc, mybir.ActivationFunctionType.Exp,
                        scale=SCALE,
                    )
                # ---- deferred tail of previous head-half / startup chores ----
                if tb == 3 and pending:
                    pending.pop(0)()
                if tb in (4, 8) and chores:
                    chores.pop(0)()
                    chores.pop(0)()
                # ---- fold tree over blocks 0..13 (14/15 deliberately left
                #      out: they join at the dns accumulation, keeping the
                #      end-of-loop critical chain short) ----
                if 7 <= tb <= 11:
                    i = tb - 7
                    nc.gpsimd.tensor_add(
                        f1[:, i, :], attnT[:, i, :], attnT[:, tb, :]
                    )
                if tb == 11:
                    nc.gpsimd.tensor_add(f2[:, 0, :], f1[:, 0, :], f1[:, 4, :])
                if tb == 12:
                    nc.vector.tensor_add(f1[:, 5, :], attnT[:, 5, :], attnT[:, 12, :])
                    nc.vector.tensor_add(f2[:, 1, :], f1[:, 1, :], f1[:, 5, :])
                    nc.vector.tensor_add(f3[:, 1, :], f2[:, 1, :], f1[:, 3, :])
                if tb == 13:
                    nc.vector.tensor_add(f1[:, 6, :], attnT[:, 6, :], attnT[:, 13, :])
                    nc.vector.tensor_add(f2[:, 2, :], f1[:, 2, :], f1[:, 6, :])
                    nc.vector.tensor_add(f3[:, 0, :], f2[:, 0, :], f2[:, 2, :])
                if tb == 14:
                    nc.vector.tensor_add(f4[:, 0, :], f3[:, 0, :], f3[:, 1, :])
            for tb in range(TB - AV_SKEW, TB):
                emit_av(tb)

            def make_tail(h=h, sh=sh, s0=s0, ys=ys, f4=f4, fl=fl):
                def tail():
                    # denominator: ones-matmul over f4 (blocks 0..14) + block 15
                    dns = psc.tile([128, SW], F32, tag="sc", name=f"dns_{h}_{sh}")
                    for c in range(2):
                        csl = slice(c * NC, (c + 1) * NC)
                        nc.tensor.matmul(
                            dns[:, csl], lhsT=ones16, rhs=f4[:, 0, csl],
                            start=True, stop=False,
                        )
                        nc.tensor.matmul(
                            dns[:, csl], lhsT=ones16, rhs=attnT[:, TB - 2, csl],
                            start=False, stop=False,
                        )
                        nc.tensor.matmul(
                            dns[:, csl], lhsT=ones16, rhs=attnT[:, TB - 1, csl],
                            start=False, stop=True,
                        )
                    for c in range(2):
                        csl = slice(c * NC, (c + 1) * NC)
                        osl = slice(s0 + c * NC, s0 + (c + 1) * NC)
                        recip = works.tile(
                            [128, NC], F32, tag="recip", name=f"recip_{h}_{sh}_{c}"
                        )
                        nc.vector.reciprocal_approx_fast(recip, dns[:, csl])
                        if h == 0:
                            nc.vector.tensor_mul(acc_a[sh][:, csl], ys[c], recip)
                            continue
                        ynorm = works.tile(
                            [128, NC], F32, tag="ynorm", name=f"yn_{h}_{sh}_{c}"
                        )
                        nc.vector.tensor_mul(ynorm, ys[c], recip)
                        if h == 1:
                            nc.gpsimd.tensor_add(
                                acc_b[sh][:, csl], acc_a[sh][:, csl], ynorm
                            )
                        elif h == 2:
                            nc.gpsimd.tensor_add(
                                acc_a[sh][:, csl], acc_b[sh][:, csl], ynorm
                            )
                        else:
                            osb = works.tile(
                                [128, NC], F32, tag="osb", name=f"osb_{h}_{sh}_{c}"
                            )
                            nc.gpsimd.tensor_add(osb, acc_a[sh][:, csl], ynorm)
                            nc.sync.dma_start(out=out_d[:, osl], in_=osb)
                return tail

            pending.append(make_tail())
    while pending:
        pending.pop(0)()

    ctx.close()


def _in_maps(inputs):
    q = np.asarray(inputs["q"], dtype=np.float32)
    Wq = np.asarray(inputs["Wq"], dtype=np.float32)
    bq = np.asarray(inputs["bq"], dtype=np.float32)
    Wk = np.asarray(inputs["Wk"], dtype=np.float32)
    Wv = np.asarray(inputs["Wv"], dtype=np.float32)
    Wo = np.asarray(inputs["Wo"], dtype=np.float32).reshape(H, E, E)
    maps = []
    for c in range(8):
        b = c // 2
        hs = slice(4 * (c % 2), 4 * (c % 2) + 4)
        maps.append(
            {
                "q": np.ascontiguousarray(q[b]),
                "Wq": np.ascontiguousarray(Wq[hs]),
                "Wk": np.ascontiguousarray(Wk[hs]),
                "Wv": np.ascontiguousarray(Wv[hs]),
                "Wo": np.ascontiguousarray(Wo[hs]),
                "bq": np.ascontiguousarray(bq[hs]),
            }
        )
    return maps


def kernel(**inputs):
    nc = build_program()
    maps = _in_maps(inputs)
    res = run_bass_kernel_spmd(nc, maps, core_ids=list(range(8)))
    bo = np.asarray(inputs["bo"], dtype=np.float32)
    bv = np.asarray(inputs["bv"], dtype=np.float32)
    Wo = np.asarray(inputs["Wo"], dtype=np.float32).reshape(H, E, E)
    # V-bias contribution folded out of the device kernel:
    # sum_h softmax(..)@ (qWv + bv) @ Wo_h = device_partials + sum_h bv_h @ Wo_h
    bo_eff = bo + np.einsum("he,hef->f", bv, Wo).astype(np.float32)
    out = np.empty((B, S, E), dtype=np.float32)
    for b in range(B):
        part = res.results[2 * b]["out"] + res.results[2 * b + 1]["out"]
        out[b] = part.T + bo_eff
    return out
